# revision 1
# baseline (speedup 1.0000x reference)
import sys, os, time
for p in ('/opt/trn_rl_repo', '/root/.axon_site/_ro/trn_rl_repo', '/root/problem'):
    if p not in sys.path and os.path.isdir(p):
        sys.path.insert(0, p)
import numpy as np

from gat_lib import Cfg, preprocess, make_consts, build, make_inputs, postprocess

LAST_EXEC_NS = None
N = 50000


def _make_runner(nc, n_cores):
    """Build the shard_map-jitted executable once (mirrors
    bass2jax.run_bass_via_pjrt) so repeated timed calls skip re-tracing."""
    import jax
    from jax.experimental.shard_map import shard_map
    from jax.sharding import Mesh, PartitionSpec
    from concourse import bass2jax, mybir
    from concourse.bass2jax import _bass_exec_p, partition_id_tensor, install_neuronx_cc_hook

    install_neuronx_cc_hook()
    partition_name = nc.partition_id_tensor.name if nc.partition_id_tensor else None
    in_names, out_names, out_avals, zero_outs = [], [], [], []
    for alloc in nc.m.functions[0].allocations:
        if not isinstance(alloc, mybir.MemoryLocationSet):
            continue
        name = alloc.memorylocations[0].name
        if alloc.kind == "ExternalInput":
            if name != partition_name:
                in_names.append(name)
        elif alloc.kind == "ExternalOutput":
            out_names.append(name)
            shape = tuple(alloc.tensor_shape)
            dtype = mybir.dt.np(alloc.dtype)
            out_avals.append(jax.core.ShapedArray(shape, dtype))
            zero_outs.append(np.zeros(shape, dtype))
    n_params = len(in_names)
    n_outs = len(out_avals)
    all_in = list(in_names) + list(out_names)
    if partition_name is not None:
        all_in.append(partition_name)
    donate = tuple(range(n_params, n_params + n_outs))

    def _body(*args):
        operands = list(args)
        if partition_name is not None:
            operands.append(partition_id_tensor())
        return tuple(
            _bass_exec_p.bind(
                *operands,
                out_avals=tuple(out_avals),
                in_names=tuple(all_in),
                out_names=tuple(out_names),
                lowering_input_output_aliases=(),
                sim_require_finite=False,
                sim_require_nnan=False,
                nc=nc,
            )
        )

    devices = jax.devices()[:n_cores]
    mesh = Mesh(np.asarray(devices), ("core",))
    in_specs = (PartitionSpec("core"),) * (n_params + n_outs)
    out_specs = (PartitionSpec("core"),) * n_outs
    sharded = jax.jit(
        shard_map(_body, mesh=mesh, in_specs=in_specs, out_specs=out_specs,
                  check_rep=False),
        donate_argnums=donate, keep_unused=True)

    def run(in_maps, n_iter=1):
        import jax
        concat_in = [
            np.concatenate([np.asarray(in_maps[c][i_name]) for c in range(n_cores)], axis=0)
            for i_name in in_names
        ]
        dev_in = jax.device_put(
            concat_in,
            [jax.sharding.NamedSharding(mesh, PartitionSpec("core"))] * n_params)
        times = []
        outs = None
        for _ in range(n_iter):
            zeros = [np.zeros((n_cores * z.shape[0], *z.shape[1:]), z.dtype)
                     for z in zero_outs]
            t0 = time.time()
            outs = sharded(*dev_in, *zeros)
            outs = [np.asarray(o) for o in outs]
            times.append(time.time() - t0)
        per_core = [
            {name: np.split(outs[i], n_cores, axis=0)[c]
             for i, name in enumerate(out_names)}
            for c in range(n_cores)
        ]
        return per_core, times

    return run


def kernel(x, edge_index, W1, att_src1, att_dst1, b1, W2, att_src2, att_dst2, b2):
    global LAST_EXEC_NS
    cfg = Cfg(N)
    t0 = time.time()
    per_core, meta = preprocess(cfg, edge_index)
    consts = make_consts(cfg, W1, att_src1, att_dst1, b1, W2, att_src2, att_dst2, b2)
    t1 = time.time()
    nc = build(cfg, meta)
    t2 = time.time()
    in_maps = make_inputs(cfg, x, per_core, consts)
    runner = _make_runner(nc, cfg.NCORES)
    n_iter = int(os.environ.get("GAT_TIME_ITERS", "1"))
    results, times = runner(in_maps, n_iter=n_iter)
    t3 = time.time()
    if os.environ.get("GAT_VERBOSE"):
        print(f"[kernel] preprocess {t1-t0:.2f}s build {t2-t1:.2f}s run {t3-t2:.2f}s")
        print(f"[kernel] per-call wall times: {[f'{x*1e3:.2f}ms' for x in times]}")
    if len(times) > 1:
        LAST_EXEC_NS = min(times[1:]) * 1e9
    out = postprocess(cfg, results)
    return np.ascontiguousarray(out.astype(np.float32))
